# revision 1
# baseline (speedup 1.0000x reference)
"""Expert-parallel MoE SwiGLU kernel for Trainium2 (8 NeuronCores).

Strategy: each of the 8 cores owns one expert's weights (w1/w3/w2).  Token
routing (the "all-to-all dispatch") is done host-side: tokens are gathered
per expert, padded to a common capacity T, and each core computes

    y_e = (silu(x_e @ w1_e) * (x_e @ w3_e)) @ w2_e          # [T, H]

for its expert's token set.  The host then scatter-adds the weighted
per-expert outputs back into the [B, H] result.  Matmuls run in float32r
(full-rate fp32 mode on the PE array); all data stays fp32 end to end.
"""

import numpy as np

_P = 128
_E = 8  # experts == cores

# (H, I, T) -> compiled Bass program
_PROG_CACHE = {}
# test hooks: set TRACE=True before calling kernel() to capture an NTFF
# profile; the BassKernelResults of the last run lands in LAST_RUN.
TRACE = False
LAST_RUN = None


def _build_program(H, I, T):
    import concourse.bass as bass
    import concourse.tile as tile
    from concourse import bacc, mybir

    f32 = mybir.dt.float32
    f32r = mybir.dt.float32r
    Sigmoid = mybir.ActivationFunctionType.Sigmoid
    ts = bass.ts

    HC = H // _P
    IC = I // _P
    assert H % _P == 0 and I % _P == 0 and T % 16 == 0

    # token free-dim tiling (both phases): chunks of <=512, as equal as
    # possible (so chunks stay >=256 and f32r matmuls keep 1 cycle/row)
    nt = -(-T // 512)
    q, r = divmod(T, nt)
    fsz = [q + (1 if i < r else 0) for i in range(nt)]
    foff = [sum(fsz[:i]) for i in range(nt)]

    nc = bacc.Bacc(
        "TRN2",
        target_bir_lowering=False,
        debug=False,
        enable_asserts=False,
        num_devices=_E,
    )
    # inputs are declared float32r (same bits as fp32 on the numpy side) so
    # the BIR verifier sees a consistent f32r producer chain into the
    # full-rate f32r matmuls
    xT = nc.dram_tensor("xT", [H, T], f32r, kind="ExternalInput").ap()
    w1 = nc.dram_tensor("w1", [H, I], f32r, kind="ExternalInput").ap()
    w3 = nc.dram_tensor("w3", [H, I], f32r, kind="ExternalInput").ap()
    w2 = nc.dram_tensor("w2", [I, H], f32r, kind="ExternalInput").ap()
    # output is y^T [H, T]: phase 2 accumulates with H on partitions so the
    # token dim needs no 128-granularity (capacity T can hug max_count)
    y = nc.dram_tensor("y", [H, T], f32, kind="ExternalOutput").ap()

    # half-block weight tiles: w_bufs//2 i-blocks of DMA lookahead
    w_bufs = 6 if T <= 544 else 4
    w2_bufs = 3

    with tile.TileContext(nc) as tc:
        with (
            tc.tile_pool(name="xp", bufs=1) as xp,
            tc.tile_pool(name="cp", bufs=1) as cp,
            tc.tile_pool(name="wp", bufs=w_bufs) as wp,
            tc.tile_pool(name="w2p", bufs=w2_bufs) as w2p,
            tc.tile_pool(name="hp", bufs=1) as hp,
            tc.tile_pool(name="sp", bufs=2) as sp,
            tc.tile_pool(name="op", bufs=4) as op,
            tc.tile_pool(name="pp", bufs=8, space="PSUM") as pp,
        ):
            zbias = cp.tile([_P, 1], f32)
            nc.any.memset(zbias[:], 0.0)

            # resident activations: x^T as [p, hc, t], h^T as [p, ic, t].
            # x loads as 4 chunks spread over different engine queues so the
            # first chunks land fast and the first accumulation group can
            # start without waiting for the whole 4MB.
            xTr = xT.rearrange("(hc p) t -> p hc t", p=_P)
            n_xc = 2 if HC % 2 == 0 else 1
            xcs = HC // n_xc
            x_engs = [nc.sync, nc.scalar]
            xs_chunks = []
            for c in range(n_xc):
                xc = xp.tile([_P, xcs, T], f32r, tag=f"xs{c}", name=f"xs_{c}")
                x_engs[c % 2].dma_start(xc[:], xTr[:, c * xcs : (c + 1) * xcs, :])
                xs_chunks.append(xc)

            def xs_slice(hc, lo, hi):
                return xs_chunks[hc // xcs][:, hc % xcs, lo:hi]

            hs = hp.tile([_P, IC, T], f32r)

            w1r = w1.rearrange("(hc p) i -> p hc i", p=_P)
            w3r = w3.rearrange("(hc p) i -> p hc i", p=_P)

            # ---- phase 1: h^T[i, t] = silu(w1^T x)[i, t] * (w3^T x)[i, t]
            # w1/w3 stream per 128-wide i-block in quarter-blocks so the PE
            # can start on the first 0.5MB and the DMA pipeline stays fine-
            # grained (each quarter is its own pool slot / dependency)
            WQ = 2 if HC % 2 == 0 else 1
            HCQ = HC // WQ
            for ic in range(IC):
                w1q = []
                w3q = []
                for qq in range(WQ):
                    w1s = wp.tile([_P, HCQ, _P], f32r, tag="w1", name=f"w1s_{ic}_{qq}")
                    nc.sync.dma_start(
                        w1s[:], w1r[:, qq * HCQ : (qq + 1) * HCQ, ts(ic, _P)]
                    )
                    w1q.append(w1s)
                    w3s = wp.tile([_P, HCQ, _P], f32r, tag="w3", name=f"w3s_{ic}_{qq}")
                    nc.scalar.dma_start(
                        w3s[:], w3r[:, qq * HCQ : (qq + 1) * HCQ, ts(ic, _P)]
                    )
                    w3q.append(w3s)
                for ti, (off, ft) in enumerate(zip(foff, fsz)):
                    pg = pp.tile([_P, 512], f32, tag="ps", name=f"pg_{ic}_{ti}")
                    pu = pp.tile([_P, 512], f32, tag="ps", name=f"pu_{ic}_{ti}")
                    for hc in range(HC):
                        nc.tensor.matmul(
                            pg[:, :ft],
                            lhsT=w1q[hc // HCQ][:, hc % HCQ, :],
                            rhs=xs_slice(hc, off, off + ft),
                            start=(hc == 0),
                            stop=(hc == HC - 1),
                        )
                    for hc in range(HC):
                        nc.tensor.matmul(
                            pu[:, :ft],
                            lhsT=w3q[hc // HCQ][:, hc % HCQ, :],
                            rhs=xs_slice(hc, off, off + ft),
                            start=(hc == 0),
                            stop=(hc == HC - 1),
                        )
                    # silu(g) * u  ==  sigmoid(g) * g * u
                    sig = sp.tile([_P, 512], f32, tag="sig", name=f"sig_{ic}_{ti}")
                    nc.scalar.activation(sig[:, :ft], pg[:, :ft], Sigmoid, bias=zbias[:])
                    gs = sp.tile([_P, 512], f32, tag="gs", name=f"gs_{ic}_{ti}")
                    nc.vector.tensor_mul(gs[:, :ft], sig[:, :ft], pg[:, :ft])
                    nc.vector.tensor_mul(
                        hs[:, ic, off : off + ft], gs[:, :ft], pu[:, :ft]
                    )

            # ---- phase 2: y^T[h, t] = sum_i w2[i, h] * h^T[i, t]
            # stationary = w2 sub-blocks [128 (i), 128 (h)], moving = h^T
            # slices; accumulate over i in PSUM with h on partitions.
            w2r = w2.rearrange("(ic p) h -> p ic h", p=_P)
            ICH = IC // 2  # stream w2 per output h-chunk in two half-blocks
            for hc2 in range(HC):
                pys = [
                    pp.tile([_P, 512], f32, tag="ps", name=f"py_{hc2}_{ti}")
                    for ti in range(nt)
                ]
                for half in range(2):
                    w2s = w2p.tile(
                        [_P, ICH, _P], f32r, tag="w2", name=f"w2s_{hc2}_{half}"
                    )
                    # alternate between the two HWDGE rings
                    dma_eng = nc.sync if (2 * hc2 + half) % 2 == 0 else nc.scalar
                    dma_eng.dma_start(
                        w2s[:], w2r[:, half * ICH : (half + 1) * ICH, ts(hc2, _P)]
                    )
                    for ich in range(ICH):
                        ic = half * ICH + ich
                        for ti, (off, ft) in enumerate(zip(foff, fsz)):
                            nc.tensor.matmul(
                                pys[ti][:, :ft],
                                lhsT=w2s[:, ich, :],
                                rhs=hs[:, ic, off : off + ft],
                                start=(ic == 0),
                                stop=(ic == IC - 1),
                            )
                for ti, (off, ft) in enumerate(zip(foff, fsz)):
                    ot = op.tile([_P, 512], f32, tag="ot", name=f"ot_{hc2}_{ti}")
                    nc.vector.tensor_copy(ot[:, :ft], pys[ti][:, :ft])
                    nc.scalar.dma_start(y[ts(hc2, _P), off : off + ft], ot[:, :ft])

    nc.compile()
    return nc


def _get_program(H, I, T):
    key = (H, I, T)
    if key not in _PROG_CACHE:
        _PROG_CACHE[key] = _build_program(H, I, T)
    return _PROG_CACHE[key]


def kernel(x, expert_indices, expert_weights, w1, w2, w3):
    global LAST_RUN
    from concourse.bass_utils import run_bass_kernel_spmd

    x = np.ascontiguousarray(np.asarray(x, dtype=np.float32))
    idx = np.asarray(expert_indices)
    idx_dtype = idx.dtype
    idx = idx.astype(np.int64)
    wts = np.asarray(expert_weights, dtype=np.float32)
    w1 = np.asarray(w1, dtype=np.float32)
    w2 = np.asarray(w2, dtype=np.float32)
    w3 = np.asarray(w3, dtype=np.float32)

    B, H = x.shape
    E, _, I = w1.shape
    assert E == _E, f"expected {_E} experts, got {E}"
    K = idx.shape[1]

    # host-side dispatch: per-token expert weight matrix (merges duplicate
    # top-k hits of the same expert), then token lists per expert
    wmat = np.zeros((B, E), np.float32)
    np.add.at(wmat, (np.arange(B)[:, None], idx), wts)
    sel = np.zeros((B, E), bool)
    sel[np.arange(B)[:, None], idx] = True

    toks = [np.nonzero(sel[:, e])[0] for e in range(E)]
    max_count = max(len(t) for t in toks)

    # capacity per round: SBUF residency (x^T and h^T tiles) caps T
    cap_limit = 608
    rounds = max(1, -(-max_count // cap_limit))
    per_round = -(-max_count // rounds)
    T = max(256, -(-per_round // 16) * 16)

    nc = _get_program(H, I, T)
    xTfull = np.ascontiguousarray(x.T)  # [H, B]

    out = np.zeros((B, H), np.float32)
    for rd in range(rounds):
        in_maps = []
        rtoks = []
        for e in range(E):
            te = toks[e][rd * per_round : (rd + 1) * per_round]
            rtoks.append(te)
            xTe = np.zeros((H, T), np.float32)
            if len(te):
                xTe[:, : len(te)] = xTfull[:, te]
            in_maps.append(
                {
                    "xT": xTe,
                    "w1": np.ascontiguousarray(w1[e]),
                    "w3": np.ascontiguousarray(w3[e]),
                    "w2": np.ascontiguousarray(w2[e]),
                }
            )
        res = run_bass_kernel_spmd(nc, in_maps, list(range(_E)), trace=TRACE)
        LAST_RUN = res
        for e in range(E):
            te = rtoks[e]
            if len(te):
                ye = res.results[e]["y"][:, : len(te)].T  # y^T [H, T] -> [n, H]
                out[te] += wmat[te, e][:, None] * ye

    return out



# revision 2
# speedup vs baseline: 1.2278x; 1.2278x over previous
"""Expert-parallel MoE SwiGLU kernel for Trainium2 (8 NeuronCores).

Strategy: each of the 8 cores owns one expert's weights (w1/w3/w2).  Token
routing (the "all-to-all dispatch") is done host-side: tokens are gathered
per expert and padded to a fixed capacity T=512, and each core computes

    y_e = (silu(x_e @ w1_e) * (x_e @ w3_e)) @ w2_e          # [T, H]

for its expert's token set.  The host scatter-adds the weighted per-expert
outputs back into the [B, H] result.  The rare overflow tokens (experts
with more than T assigned tokens) are computed host-side in fp32.

All matmul operands are bf16 (PSUM accumulation stays fp32), which halves
HBM weight traffic vs fp32 and keeps the PE at 1 cycle/row for any moving
size.  Weights are host-repacked into PE-ready tiled layouts so every DMA
line is 4-8KB contiguous per partition.
"""

import numpy as np

_P = 128
_E = 8   # experts == cores
_T = 512  # fixed device token capacity per expert (1 PSUM bank of fp32)

_PROG_CACHE = {}
# test hooks: set TRACE=True before calling kernel() to capture an NTFF
# profile; the BassKernelResults of the last run lands in LAST_RUN.
TRACE = False
LAST_RUN = None


def _build_program(H, I):
    import concourse.bass as bass
    import concourse.tile as tile
    from concourse import bacc, mybir

    f32 = mybir.dt.float32
    bf16 = mybir.dt.bfloat16
    Silu = mybir.ActivationFunctionType.Silu
    ts = bass.ts

    T = _T
    HC = H // _P   # 16 contraction blocks for phase 1
    IC = I // _P   # 32 contraction blocks for phase 2
    assert H % _P == 0 and I % _P == 0

    nc = bacc.Bacc(
        "TRN2",
        target_bir_lowering=False,
        debug=False,
        enable_asserts=False,
        num_devices=_E,
    )
    # Host-pretiled layouts (per-partition lines are fully contiguous):
    #   x  [P, HC, T]     x[p, hc, t]  = x_tok[t, hc*P + p]          (bf16)
    #   w1 [IC*P, HC, P]  w1[ic*P+p, hc, j] = w1[hc*P+p, ic*P+j]     (bf16)
    #   w3 same as w1
    #   w2 [HC*P, IC, P]  w2[hc*P+p, ic, j] = w2[ic*P+p, hc*P+j]     (bf16)
    #   y  [H, T]         y[h, t] (fp32), h on partitions per block
    x = nc.dram_tensor("x", [_P, HC, T], bf16, kind="ExternalInput").ap()
    w1 = nc.dram_tensor("w1", [IC * _P, HC, _P], bf16, kind="ExternalInput").ap()
    w3 = nc.dram_tensor("w3", [IC * _P, HC, _P], bf16, kind="ExternalInput").ap()
    w2 = nc.dram_tensor("w2", [HC * _P, IC, _P], bf16, kind="ExternalInput").ap()
    y = nc.dram_tensor("y", [H, T], f32, kind="ExternalOutput").ap()

    NXC = 4           # x loads in 4 chunks so the first matmuls start early
    XW = HC // NXC

    with tile.TileContext(nc) as tc:
        with (
            tc.tile_pool(name="xp", bufs=1) as xp,
            tc.tile_pool(name="cp", bufs=1) as cp,
            tc.tile_pool(name="wp", bufs=8) as wp,
            tc.tile_pool(name="w2p", bufs=4) as w2p,
            tc.tile_pool(name="hp", bufs=1) as hp,
            tc.tile_pool(name="sp", bufs=2) as sp,
            tc.tile_pool(name="op", bufs=4) as op,
            tc.tile_pool(name="pp", bufs=8, space="PSUM") as pp,
        ):
            zbias = cp.tile([_P, 1], f32)
            nc.any.memset(zbias[:], 0.0)

            # resident x^T [p, hc, t]; chunks split across both HWDGE queues
            xcs = []
            for c in range(NXC):
                xc = xp.tile([_P, XW, T], bf16, tag=f"x{c}", name=f"x_{c}")
                eng = nc.scalar if c % 2 == 0 else nc.sync
                eng.dma_start(xc[:], x[:, c * XW : (c + 1) * XW, :])
                xcs.append(xc)

            def xsl(hc):
                return xcs[hc // XW][:, hc % XW, :]

            # resident h^T [p(i), ic, t] in bf16
            hs = hp.tile([_P, IC, T], bf16)

            # ---- phase 1: h^T[i, t] = silu(w1^T x)[i, t] * (w3^T x)[i, t]
            for ic in range(IC):
                w1s = wp.tile([_P, HC, _P], bf16, tag="w1", name=f"w1_{ic}")
                nc.sync.dma_start(w1s[:], w1[ts(ic, _P), :, :])
                w3s = wp.tile([_P, HC, _P], bf16, tag="w3", name=f"w3_{ic}")
                # scalar's queue starts with 2 x chunks; keep the first w3
                # on sync so the ic=0 up-group is not starved behind x
                w3_eng = nc.sync if ic == 0 else nc.scalar
                w3_eng.dma_start(w3s[:], w3[ts(ic, _P), :, :])

                pg = pp.tile([_P, T], f32, tag="ps", name=f"pg_{ic}")
                pu = pp.tile([_P, T], f32, tag="ps", name=f"pu_{ic}")
                for hc in range(HC):
                    nc.tensor.matmul(
                        pg[:],
                        lhsT=w1s[:, hc, :],
                        rhs=xsl(hc),
                        start=(hc == 0),
                        stop=(hc == HC - 1),
                    )
                for hc in range(HC):
                    nc.tensor.matmul(
                        pu[:],
                        lhsT=w3s[:, hc, :],
                        rhs=xsl(hc),
                        start=(hc == 0),
                        stop=(hc == HC - 1),
                    )
                sg = sp.tile([_P, T], f32, tag="sg", name=f"sg_{ic}")
                nc.scalar.activation(sg[:], pg[:], Silu, bias=zbias[:])
                nc.vector.tensor_mul(hs[:, ic, :], sg[:], pu[:])

            # ---- phase 2: y[h, t] = sum_i w2[i, h] * h^T[i, t]
            # w2 tiles stream on both queues; pool lookahead (bufs=4) makes
            # the first tiles prefetch during phase 1's tail.
            for hc2 in range(HC):
                w2s = w2p.tile([_P, IC, _P], bf16, tag="w2", name=f"w2_{hc2}")
                w2_eng = nc.sync if hc2 % 2 == 0 else nc.scalar
                w2_eng.dma_start(w2s[:], w2[ts(hc2, _P), :, :])

                py = pp.tile([_P, T], f32, tag="ps", name=f"py_{hc2}")
                for ic in range(IC):
                    nc.tensor.matmul(
                        py[:],
                        lhsT=w2s[:, ic, :],
                        rhs=hs[:, ic, :],
                        start=(ic == 0),
                        stop=(ic == IC - 1),
                    )
                ot = op.tile([_P, T], f32, tag="ot", name=f"ot_{hc2}")
                nc.vector.tensor_copy(ot[:], py[:])
                out_eng = nc.scalar if hc2 % 2 == 0 else nc.sync
                out_eng.dma_start(y[ts(hc2, _P), :], ot[:])

    nc.compile()
    return nc


def _get_program(H, I):
    key = (H, I)
    if key not in _PROG_CACHE:
        _PROG_CACHE[key] = _build_program(H, I)
    return _PROG_CACHE[key]


def kernel(x, expert_indices, expert_weights, w1, w2, w3):
    global LAST_RUN
    import ml_dtypes
    from concourse.bass_utils import run_bass_kernel_spmd

    bf16 = ml_dtypes.bfloat16
    x = np.ascontiguousarray(np.asarray(x, dtype=np.float32))
    idx = np.asarray(expert_indices).astype(np.int64)
    wts = np.asarray(expert_weights, dtype=np.float32)
    w1 = np.asarray(w1, dtype=np.float32)
    w2 = np.asarray(w2, dtype=np.float32)
    w3 = np.asarray(w3, dtype=np.float32)

    B, H = x.shape
    E, _, I = w1.shape
    assert E == _E, f"expected {_E} experts, got {E}"
    HC, IC, P, T = H // _P, I // _P, _P, _T

    # host-side dispatch: per-token expert weight matrix (merges duplicate
    # top-k hits of the same expert), then token lists per expert
    wmat = np.zeros((B, E), np.float32)
    np.add.at(wmat, (np.arange(B)[:, None], idx), wts)
    sel = np.zeros((B, E), bool)
    sel[np.arange(B)[:, None], idx] = True
    toks = [np.nonzero(sel[:, e])[0] for e in range(E)]

    nc = _get_program(H, I)

    xb = x.astype(bf16)
    in_maps = []
    for e in range(E):
        te = toks[e][:T]
        n = len(te)
        xe = np.zeros((P, HC, T), bf16)
        if n:
            xe[:, :, :n] = xb[te].reshape(n, HC, P).transpose(2, 1, 0)
        w1e = w1[e].astype(bf16).reshape(HC, P, IC, P).transpose(2, 1, 0, 3)
        w3e = w3[e].astype(bf16).reshape(HC, P, IC, P).transpose(2, 1, 0, 3)
        w2e = w2[e].astype(bf16).reshape(IC, P, HC, P).transpose(2, 1, 0, 3)
        in_maps.append(
            {
                "x": np.ascontiguousarray(xe),
                "w1": np.ascontiguousarray(w1e).reshape(IC * P, HC, P),
                "w3": np.ascontiguousarray(w3e).reshape(IC * P, HC, P),
                "w2": np.ascontiguousarray(w2e).reshape(HC * P, IC, P),
            }
        )
    res = run_bass_kernel_spmd(nc, in_maps, list(range(_E)), trace=TRACE)
    LAST_RUN = res

    out = np.zeros((B, H), np.float32)
    for e in range(E):
        te = toks[e][:T]
        if len(te):
            ye = res.results[e]["y"][:, : len(te)].T  # [H, T] -> [n, H]
            out[te] += wmat[te, e][:, None] * ye
        ov = toks[e][T:]  # overflow tokens: exact fp32 on host
        if len(ov):
            xo = x[ov]
            g = xo @ w1[e]
            u = xo @ w3[e]
            h = (g / (1.0 + np.exp(-g))) * u
            out[ov] += wmat[ov, e][:, None] * (h @ w2[e])
    return out


# revision 5
# speedup vs baseline: 1.3196x; 1.0748x over previous
"""Expert-parallel MoE SwiGLU kernel for Trainium2 (8 NeuronCores).

Strategy: each of the 8 cores owns one expert's weights (w1/w3/w2).  Token
routing (the "all-to-all dispatch") is done host-side: tokens are gathered
per expert and padded to a fixed capacity T=512, and each core computes

    y_e = (silu(x_e @ w1_e) * (x_e @ w3_e)) @ w2_e          # [T, H]

for its expert's token set.  The host scatter-adds the weighted per-expert
outputs back into the [B, H] result.  The rare overflow tokens (experts
with more than T assigned tokens) are computed host-side in fp32.

All matmul operands are bf16 (PSUM accumulation stays fp32), which halves
HBM weight traffic vs fp32 and keeps the PE at 1 cycle/row for any moving
size.  Weights are host-repacked into PE-ready tiled layouts so every DMA
line is 4-8KB contiguous per partition.
"""

import numpy as np

_P = 128
_E = 8   # experts == cores
# fixed device token capacity per expert: B*K/E pairs minus expected top-k
# duplicate merges lands at ~480; the rare overflow tokens of hotter experts
# are computed host-side in exact fp32
_T = 480

_PROG_CACHE = {}
# test hooks: set TRACE=True before calling kernel() to capture an NTFF
# profile; the BassKernelResults of the last run lands in LAST_RUN.
TRACE = False
LAST_RUN = None


def _build_program(H, I):
    import concourse.bass as bass
    import concourse.tile as tile
    from concourse import bacc, mybir

    f32 = mybir.dt.float32
    bf16 = mybir.dt.bfloat16
    Silu = mybir.ActivationFunctionType.Silu
    ts = bass.ts

    T = _T
    HC = H // _P   # 16 contraction blocks for phase 1
    IC = I // _P   # 32 contraction blocks for phase 2
    assert H % _P == 0 and I % _P == 0

    nc = bacc.Bacc(
        "TRN2",
        target_bir_lowering=False,
        debug=False,
        enable_asserts=False,
        num_devices=_E,
    )
    # Host-pretiled layouts (per-partition lines are fully contiguous):
    #   x  [P, HC, T]     x[p, hc, t]  = x_tok[t, hc*P + p]          (bf16)
    #   w1 [IC*P, HC, P]  w1[ic*P+p, hc, j] = w1[hc*P+p, ic*P+j]     (bf16)
    #   w3 same as w1
    #   w2 [HC*P, IC, P]  w2[hc*P+p, ic, j] = w2[ic*P+p, hc*P+j]     (bf16)
    #   y  [H, T]         y[h, t] (fp32), h on partitions per block
    x = nc.dram_tensor("x", [_P, HC, T], bf16, kind="ExternalInput").ap()
    w1 = nc.dram_tensor("w1", [IC * _P, HC, _P], bf16, kind="ExternalInput").ap()
    w3 = nc.dram_tensor("w3", [IC * _P, HC, _P], bf16, kind="ExternalInput").ap()
    w2 = nc.dram_tensor("w2", [HC * _P, IC, _P], bf16, kind="ExternalInput").ap()
    y = nc.dram_tensor("y", [H, T], f32, kind="ExternalOutput").ap()

    NXC = 4           # x loads in 4 chunks so the first matmuls start early
    XW = HC // NXC

    with tile.TileContext(nc) as tc:
        with (
            tc.tile_pool(name="xp", bufs=1) as xp,
            tc.tile_pool(name="cp", bufs=1) as cp,
            tc.tile_pool(name="wp", bufs=8) as wp,
            tc.tile_pool(name="w2p", bufs=4) as w2p,
            tc.tile_pool(name="hp", bufs=1) as hp,
            tc.tile_pool(name="sp", bufs=2) as sp,
            tc.tile_pool(name="op", bufs=4) as op,
            tc.tile_pool(name="pp", bufs=8, space="PSUM") as pp,
        ):
            zbias = cp.tile([_P, 1], f32)
            nc.any.memset(zbias[:], 0.0)

            # startup critical path: w1[0] leads the sync queue while x0
            # leads scalar, so the first gate matmuls start as early as
            # possible; remaining x chunks and w3[0] follow right behind
            w1s0 = wp.tile([_P, HC, _P], bf16, tag="w1", name="w1_0")
            nc.sync.dma_start(w1s0[:], w1[ts(0, _P), :, :])
            xcs = []
            for c in range(NXC):
                xc = xp.tile([_P, XW, T], bf16, tag=f"x{c}", name=f"x_{c}")
                eng = nc.scalar if c % 2 == 0 else nc.sync
                eng.dma_start(xc[:], x[:, c * XW : (c + 1) * XW, :])
                xcs.append(xc)
            w3s0 = wp.tile([_P, HC, _P], bf16, tag="w3", name="w3_0")
            nc.scalar.dma_start(w3s0[:], w3[ts(0, _P), :, :])

            def xsl(hc):
                return xcs[hc // XW][:, hc % XW, :]

            # resident h^T [p(i), ic, t] in bf16
            hs = hp.tile([_P, IC, T], bf16)

            # ---- phase 1: h^T[i, t] = silu(w1^T x)[i, t] * (w3^T x)[i, t]
            for ic in range(IC):
                if ic == 0:
                    w1s, w3s = w1s0, w3s0
                else:
                    w1s = wp.tile([_P, HC, _P], bf16, tag="w1", name=f"w1_{ic}")
                    nc.sync.dma_start(w1s[:], w1[ts(ic, _P), :, :])
                    w3s = wp.tile([_P, HC, _P], bf16, tag="w3", name=f"w3_{ic}")
                    nc.scalar.dma_start(w3s[:], w3[ts(ic, _P), :, :])

                pg = pp.tile([_P, T], f32, tag="ps", name=f"pg_{ic}")
                pu = pp.tile([_P, T], f32, tag="ps", name=f"pu_{ic}")
                for hc in range(HC):
                    nc.tensor.matmul(
                        pg[:],
                        lhsT=w1s[:, hc, :],
                        rhs=xsl(hc),
                        start=(hc == 0),
                        stop=(hc == HC - 1),
                    )
                for hc in range(HC):
                    nc.tensor.matmul(
                        pu[:],
                        lhsT=w3s[:, hc, :],
                        rhs=xsl(hc),
                        start=(hc == 0),
                        stop=(hc == HC - 1),
                    )
                sg = sp.tile([_P, T], f32, tag="sg", name=f"sg_{ic}")
                nc.scalar.activation(sg[:], pg[:], Silu, bias=zbias[:])
                nc.vector.tensor_mul(hs[:, ic, :], sg[:], pu[:])

            # ---- phase 2: y[h, t] = sum_i w2[i, h] * h^T[i, t]
            # w2 tiles stream on both queues; pool lookahead (bufs=4) makes
            # the first tiles prefetch during phase 1's tail.
            for hc2 in range(HC):
                w2s = w2p.tile([_P, IC, _P], bf16, tag="w2", name=f"w2_{hc2}")
                w2_eng = nc.sync if hc2 % 2 == 0 else nc.scalar
                w2_eng.dma_start(w2s[:], w2[ts(hc2, _P), :, :])

                py = pp.tile([_P, T], f32, tag="ps", name=f"py_{hc2}")
                for ic in range(IC):
                    nc.tensor.matmul(
                        py[:],
                        lhsT=w2s[:, ic, :],
                        rhs=hs[:, ic, :],
                        start=(ic == 0),
                        stop=(ic == IC - 1),
                    )
                # write back in half-column chunks on both queues so the
                # final block's drain is pipelined
                ot = op.tile([_P, T], f32, tag="ot", name=f"ot_{hc2}")
                TH = T // 2
                for half in range(2):
                    sl = slice(half * TH, (half + 1) * TH)
                    nc.vector.tensor_copy(ot[:, sl], py[:, sl])
                    out_eng = nc.scalar if (hc2 + half) % 2 == 0 else nc.sync
                    out_eng.dma_start(y[ts(hc2, _P), sl], ot[:, sl])

    nc.compile()
    return nc


def _get_program(H, I):
    key = (H, I)
    if key not in _PROG_CACHE:
        _PROG_CACHE[key] = _build_program(H, I)
    return _PROG_CACHE[key]


def kernel(x, expert_indices, expert_weights, w1, w2, w3):
    global LAST_RUN
    import ml_dtypes
    from concourse.bass_utils import run_bass_kernel_spmd

    bf16 = ml_dtypes.bfloat16
    x = np.ascontiguousarray(np.asarray(x, dtype=np.float32))
    idx = np.asarray(expert_indices).astype(np.int64)
    wts = np.asarray(expert_weights, dtype=np.float32)
    w1 = np.asarray(w1, dtype=np.float32)
    w2 = np.asarray(w2, dtype=np.float32)
    w3 = np.asarray(w3, dtype=np.float32)

    B, H = x.shape
    E, _, I = w1.shape
    assert E == _E, f"expected {_E} experts, got {E}"
    HC, IC, P, T = H // _P, I // _P, _P, _T

    # host-side dispatch: per-token expert weight matrix (merges duplicate
    # top-k hits of the same expert), then token lists per expert
    wmat = np.zeros((B, E), np.float32)
    np.add.at(wmat, (np.arange(B)[:, None], idx), wts)
    sel = np.zeros((B, E), bool)
    sel[np.arange(B)[:, None], idx] = True
    toks = [np.nonzero(sel[:, e])[0] for e in range(E)]

    nc = _get_program(H, I)

    xb = x.astype(bf16)
    in_maps = []
    for e in range(E):
        te = toks[e][:T]
        n = len(te)
        xe = np.zeros((P, HC, T), bf16)
        if n:
            xe[:, :, :n] = xb[te].reshape(n, HC, P).transpose(2, 1, 0)
        w1e = w1[e].astype(bf16).reshape(HC, P, IC, P).transpose(2, 1, 0, 3)
        w3e = w3[e].astype(bf16).reshape(HC, P, IC, P).transpose(2, 1, 0, 3)
        w2e = w2[e].astype(bf16).reshape(IC, P, HC, P).transpose(2, 1, 0, 3)
        in_maps.append(
            {
                "x": np.ascontiguousarray(xe),
                "w1": np.ascontiguousarray(w1e).reshape(IC * P, HC, P),
                "w3": np.ascontiguousarray(w3e).reshape(IC * P, HC, P),
                "w2": np.ascontiguousarray(w2e).reshape(HC * P, IC, P),
            }
        )
    res = run_bass_kernel_spmd(nc, in_maps, list(range(_E)), trace=TRACE)
    LAST_RUN = res

    out = np.zeros((B, H), np.float32)
    for e in range(E):
        te = toks[e][:T]
        if len(te):
            ye = res.results[e]["y"][:, : len(te)].T  # [H, T] -> [n, H]
            out[te] += wmat[te, e][:, None] * ye
        ov = toks[e][T:]  # overflow tokens: exact fp32 on host
        if len(ov):
            xo = x[ov]
            g = xo @ w1[e]
            u = xo @ w3[e]
            h = (g / (1.0 + np.exp(-g))) * u
            out[ov] += wmat[ov, e][:, None] * (h @ w2[e])
    return out


# revision 7
# speedup vs baseline: 1.3300x; 1.0079x over previous
"""Expert-parallel MoE SwiGLU kernel for Trainium2 (8 NeuronCores).

Strategy: each of the 8 cores owns one expert's weights (w1/w3/w2).  Token
routing (the "all-to-all dispatch") is done host-side: tokens are gathered
per expert and padded to a fixed capacity T=512, and each core computes

    y_e = (silu(x_e @ w1_e) * (x_e @ w3_e)) @ w2_e          # [T, H]

for its expert's token set.  The host scatter-adds the weighted per-expert
outputs back into the [B, H] result.  The rare overflow tokens (experts
with more than T assigned tokens) are computed host-side in fp32.

All matmul operands are bf16 (PSUM accumulation stays fp32), which halves
HBM weight traffic vs fp32 and keeps the PE at 1 cycle/row for any moving
size.  Weights are host-repacked into PE-ready tiled layouts so every DMA
line is 4-8KB contiguous per partition.
"""

import numpy as np

_P = 128
_E = 8   # experts == cores
# fixed device token capacity per expert: B*K/E pairs minus expected top-k
# duplicate merges lands at ~480; the rare overflow tokens of hotter experts
# are computed host-side in exact fp32
_T = 480

_PROG_CACHE = {}
# test hooks: set TRACE=True before calling kernel() to capture an NTFF
# profile; the BassKernelResults of the last run lands in LAST_RUN.
TRACE = False
LAST_RUN = None


def _build_program(H, I):
    import concourse.bass as bass
    import concourse.tile as tile
    from concourse import bacc, mybir

    f32 = mybir.dt.float32
    bf16 = mybir.dt.bfloat16
    Silu = mybir.ActivationFunctionType.Silu
    ts = bass.ts

    T = _T
    HC = H // _P   # 16 contraction blocks for phase 1
    IC = I // _P   # 32 contraction blocks for phase 2
    assert H % _P == 0 and I % _P == 0

    nc = bacc.Bacc(
        "TRN2",
        target_bir_lowering=False,
        debug=False,
        enable_asserts=False,
        num_devices=_E,
    )
    # Host-pretiled layouts (per-partition lines are fully contiguous):
    #   x  [P, HC, T]     x[p, hc, t]  = x_tok[t, hc*P + p]          (bf16)
    #   w1 [IC*P, HC, P]  w1[ic*P+p, hc, j] = w1[hc*P+p, ic*P+j]     (bf16)
    #   w3 same as w1
    #   w2 [HC*P, IC, P]  w2[hc*P+p, ic, j] = w2[ic*P+p, hc*P+j]     (bf16)
    #   y  [H, T]         y[h, t] (fp32), h on partitions per block
    x = nc.dram_tensor("x", [_P, HC, T], bf16, kind="ExternalInput").ap()
    w1 = nc.dram_tensor("w1", [IC * _P, HC, _P], bf16, kind="ExternalInput").ap()
    w3 = nc.dram_tensor("w3", [IC * _P, HC, _P], bf16, kind="ExternalInput").ap()
    w2 = nc.dram_tensor("w2", [HC * _P, IC, _P], bf16, kind="ExternalInput").ap()
    y = nc.dram_tensor("y", [H, T], f32, kind="ExternalOutput").ap()

    NXC = 4           # x loads in 4 chunks so the first matmuls start early
    XW = HC // NXC

    with tile.TileContext(nc) as tc:
        with (
            tc.tile_pool(name="xp", bufs=1) as xp,
            tc.tile_pool(name="cp", bufs=1) as cp,
            tc.tile_pool(name="wp", bufs=8) as wp,
            tc.tile_pool(name="w2p", bufs=4) as w2p,
            tc.tile_pool(name="hp", bufs=1) as hp,
            tc.tile_pool(name="sp", bufs=2) as sp,
            tc.tile_pool(name="op", bufs=4) as op,
            tc.tile_pool(name="pp", bufs=7, space="PSUM") as pp,
            tc.tile_pool(name="wup", bufs=1, space="PSUM") as wup,
        ):
            zbias = cp.tile([_P, 1], f32)
            nc.any.memset(zbias[:], 0.0)

            # warm up the PE p-state during the initial x/w1 DMA wait with
            # throwaway matmuls on memset tiles, so the first real matmuls
            # run at full clock
            wl = cp.tile([_P, _P], bf16, tag="wl")
            nc.vector.memset(wl[:], 0.0)
            wr = cp.tile([_P, T], bf16, tag="wr")
            nc.gpsimd.memset(wr[:], 0.0)
            wud = wup.tile([_P, T], f32, tag="wu")
            for _ in range(10):
                nc.tensor.matmul(
                    wud[:], lhsT=wl[:], rhs=wr[:], start=True, stop=True
                )

            # startup critical path: w1[0] leads the sync queue while x0
            # leads scalar, so the first gate matmuls start as early as
            # possible; remaining x chunks and w3[0] follow right behind
            w1s0 = wp.tile([_P, HC, _P], bf16, tag="w1", name="w1_0")
            nc.sync.dma_start(w1s0[:], w1[ts(0, _P), :, :])
            xcs = []
            for c in range(NXC):
                xc = xp.tile([_P, XW, T], bf16, tag=f"x{c}", name=f"x_{c}")
                eng = nc.scalar if c % 2 == 0 else nc.sync
                eng.dma_start(xc[:], x[:, c * XW : (c + 1) * XW, :])
                xcs.append(xc)
            w3s0 = wp.tile([_P, HC, _P], bf16, tag="w3", name="w3_0")
            nc.scalar.dma_start(w3s0[:], w3[ts(0, _P), :, :])

            def xsl(hc):
                return xcs[hc // XW][:, hc % XW, :]

            # resident h^T [p(i), ic, t] in bf16
            hs = hp.tile([_P, IC, T], bf16)

            # ---- phase 1: h^T[i, t] = silu(w1^T x)[i, t] * (w3^T x)[i, t]
            for ic in range(IC):
                if ic == 0:
                    w1s, w3s = w1s0, w3s0
                else:
                    w1s = wp.tile([_P, HC, _P], bf16, tag="w1", name=f"w1_{ic}")
                    nc.sync.dma_start(w1s[:], w1[ts(ic, _P), :, :])
                    w3s = wp.tile([_P, HC, _P], bf16, tag="w3", name=f"w3_{ic}")
                    nc.scalar.dma_start(w3s[:], w3[ts(ic, _P), :, :])

                pg = pp.tile([_P, T], f32, tag="ps", name=f"pg_{ic}")
                pu = pp.tile([_P, T], f32, tag="ps", name=f"pu_{ic}")
                for hc in range(HC):
                    nc.tensor.matmul(
                        pg[:],
                        lhsT=w1s[:, hc, :],
                        rhs=xsl(hc),
                        start=(hc == 0),
                        stop=(hc == HC - 1),
                    )
                for hc in range(HC):
                    nc.tensor.matmul(
                        pu[:],
                        lhsT=w3s[:, hc, :],
                        rhs=xsl(hc),
                        start=(hc == 0),
                        stop=(hc == HC - 1),
                    )
                sg = sp.tile([_P, T], f32, tag="sg", name=f"sg_{ic}")
                nc.scalar.activation(sg[:], pg[:], Silu, bias=zbias[:])
                nc.vector.tensor_mul(hs[:, ic, :], sg[:], pu[:])

            # ---- phase 2: y[h, t] = sum_i w2[i, h] * h^T[i, t]
            # w2 tiles stream on both queues; pool lookahead (bufs=4) makes
            # the first tiles prefetch during phase 1's tail.
            for hc2 in range(HC):
                w2s = w2p.tile([_P, IC, _P], bf16, tag="w2", name=f"w2_{hc2}")
                w2_eng = nc.sync if hc2 % 2 == 0 else nc.scalar
                w2_eng.dma_start(w2s[:], w2[ts(hc2, _P), :, :])

                # the last block accumulates its column halves as separate
                # PSUM groups so the first half's writeback overlaps the
                # second half's matmuls, shortening the drain
                TH = T // 2
                col_groups = (
                    [slice(0, T)] if hc2 < HC - 1 else [slice(0, TH), slice(TH, T)]
                )
                py = pp.tile([_P, T], f32, tag="ps", name=f"py_{hc2}")
                ot = op.tile([_P, T], f32, tag="ot", name=f"ot_{hc2}")
                for cg in col_groups:
                    for ic in range(IC):
                        nc.tensor.matmul(
                            py[:, cg],
                            lhsT=w2s[:, ic, :],
                            rhs=hs[:, ic, cg],
                            start=(ic == 0),
                            stop=(ic == IC - 1),
                        )
                    # write back in half-column chunks on both queues so
                    # every block's drain is pipelined
                    for half in ([0, 1] if cg.stop - cg.start == T else [0]):
                        sl = (
                            slice(half * TH, (half + 1) * TH)
                            if cg.stop - cg.start == T
                            else cg
                        )
                        nc.vector.tensor_copy(ot[:, sl], py[:, sl])
                        out_eng = nc.scalar if (hc2 + half) % 2 == 0 else nc.sync
                        out_eng.dma_start(y[ts(hc2, _P), sl], ot[:, sl])

    nc.compile()
    return nc


def _get_program(H, I):
    key = (H, I)
    if key not in _PROG_CACHE:
        _PROG_CACHE[key] = _build_program(H, I)
    return _PROG_CACHE[key]


def kernel(x, expert_indices, expert_weights, w1, w2, w3):
    global LAST_RUN
    import ml_dtypes
    from concourse.bass_utils import run_bass_kernel_spmd

    bf16 = ml_dtypes.bfloat16
    x = np.ascontiguousarray(np.asarray(x, dtype=np.float32))
    idx = np.asarray(expert_indices).astype(np.int64)
    wts = np.asarray(expert_weights, dtype=np.float32)
    w1 = np.asarray(w1, dtype=np.float32)
    w2 = np.asarray(w2, dtype=np.float32)
    w3 = np.asarray(w3, dtype=np.float32)

    B, H = x.shape
    E, _, I = w1.shape
    assert E == _E, f"expected {_E} experts, got {E}"
    HC, IC, P, T = H // _P, I // _P, _P, _T

    # host-side dispatch: per-token expert weight matrix (merges duplicate
    # top-k hits of the same expert), then token lists per expert
    wmat = np.zeros((B, E), np.float32)
    np.add.at(wmat, (np.arange(B)[:, None], idx), wts)
    sel = np.zeros((B, E), bool)
    sel[np.arange(B)[:, None], idx] = True
    toks = [np.nonzero(sel[:, e])[0] for e in range(E)]

    nc = _get_program(H, I)

    xb = x.astype(bf16)
    in_maps = []
    for e in range(E):
        te = toks[e][:T]
        n = len(te)
        xe = np.zeros((P, HC, T), bf16)
        if n:
            xe[:, :, :n] = xb[te].reshape(n, HC, P).transpose(2, 1, 0)
        w1e = w1[e].astype(bf16).reshape(HC, P, IC, P).transpose(2, 1, 0, 3)
        w3e = w3[e].astype(bf16).reshape(HC, P, IC, P).transpose(2, 1, 0, 3)
        w2e = w2[e].astype(bf16).reshape(IC, P, HC, P).transpose(2, 1, 0, 3)
        in_maps.append(
            {
                "x": np.ascontiguousarray(xe),
                "w1": np.ascontiguousarray(w1e).reshape(IC * P, HC, P),
                "w3": np.ascontiguousarray(w3e).reshape(IC * P, HC, P),
                "w2": np.ascontiguousarray(w2e).reshape(HC * P, IC, P),
            }
        )
    res = run_bass_kernel_spmd(nc, in_maps, list(range(_E)), trace=TRACE)
    LAST_RUN = res

    out = np.zeros((B, H), np.float32)
    for e in range(E):
        te = toks[e][:T]
        if len(te):
            ye = res.results[e]["y"][:, : len(te)].T  # [H, T] -> [n, H]
            out[te] += wmat[te, e][:, None] * ye
        ov = toks[e][T:]  # overflow tokens: exact fp32 on host
        if len(ov):
            xo = x[ov]
            g = xo @ w1[e]
            u = xo @ w3[e]
            h = (g / (1.0 + np.exp(-g))) * u
            out[ov] += wmat[ov, e][:, None] * (h @ w2[e])
    return out
